# revision 3
# baseline (speedup 1.0000x reference)
"""Bahdanau attention kernel v2 for 8 TRN2 NeuronCores.

Math: scores[q,k] = w2 . tanh(qW[q,:] + b1 + kW[k,:])  (b2 dropped, softmax-
invariant).  tanh(x+y) is approximated by a collapsed separable expansion

    tanh(x+y) ~= g(x) + sum_j Phi_j(x) * tanh(1.5*(y - mu_j)),   j = 1..4

where Phi_j = sum_i C[i,j] * phi_i(x) are linear combos (computed on the
VectorE) of a small x-dictionary {x, tanh(1.5(x-mu))} evaluated by the
ScalarE.  g(x) is softmax-invariant and dropped.  All activations (tanh,
exp) live in the single `exp_and_others` table set: one ACT_TABLE_LOAD,
triggered by a dummy act at t=0 so it overlaps the input DMA.

scores = sum_j (w2*Phi_j)^T @ psi_j is a TensorE contraction (16 matmuls),
followed by exp-softmax (mask via host-sent 1-mask) and the context matmul
on the transposed normalized weights.

Sharding: data-parallel, core = (batch b, query-half qh); each core computes
a [128, 512] block of weights and context.  Outputs are bf16, upcast on host.
"""

import numpy as np
import ml_dtypes

from contextlib import ExitStack
from concourse import bass, bacc, tile, mybir
from concourse.bass_utils import run_bass_kernel_spmd

BF16 = mybir.dt.bfloat16
F32 = mybir.dt.float32
AF = mybir.ActivationFunctionType
OP = mybir.AluOpType
NPBF = ml_dtypes.bfloat16
F8 = mybir.dt.float8e4
NPF8 = ml_dtypes.float8_e4m3

B, Q, K, H, A = 4, 256, 512, 512, 512
QSH = 128
N_CORES = 8

ALPHA = 1.5
# x-dictionary: index -> spec. 0='one' (folded), 1='lin' (x0 itself),
# 2..8 = tanh(1.5*(x-mu)), mus:
XMUS = {2: -0.9, 3: -0.45, 4: 0.0, 5: 0.45, 6: 0.9}
YMUS = [-0.8, 0.0, 0.8]
NY = len(YMUS)
# C[0, j] ('one' row): folded into the w2-fold pass
C0 = [0.17376022026341648, -0.08117130489640326, 0.2138754878123161]
# (x_index, y_col, coeff)
ENTRIES = [
    (5, 0, 0.8101097519208896),
    (6, 0, -0.6013097045398987),
    (3, 1, 0.519418159350108),
    (5, 1, -0.5298784785625438),
    (1, 2, 0.05761974496291286),
    (2, 2, 0.5656294633784432),
    (3, 2, -0.8359265955681372),
]
XUSED = sorted({i for i, _, _ in ENTRIES if i >= 2})
XB0 = 9
YB0 = XB0 + len(XUSED)


def _build_kernel():
    nc = bacc.Bacc("TRN2", target_bir_lowering=False, debug=False,
                   num_devices=N_CORES)

    d_xin = nc.declare_dram_parameter("xin", [128, 2560], F8, isOutput=False)
    d_cst = nc.declare_dram_parameter("cst", [128, 19], F32, isOutput=False)
    d_yin = nc.declare_dram_parameter("yin", [128, 4096], F8, isOutput=False)
    d_tail = nc.declare_dram_parameter("tl", [128, 2688], BF16, isOutput=False)
    d_wout = nc.declare_dram_parameter("wout", [QSH, K], BF16, isOutput=True)
    d_cout = nc.declare_dram_parameter("cout", [QSH, H], BF16, isOutput=True)

    with tile.TileContext(nc) as tc, ExitStack() as ctx:
        sb = ctx.enter_context(tc.tile_pool(name="sb", bufs=1))
        ps_q = ctx.enter_context(tc.tile_pool(name="psq", bufs=1, space="PSUM"))
        ps_k = ctx.enter_context(tc.tile_pool(name="psk", bufs=1, space="PSUM"))
        ps_s = ctx.enter_context(tc.tile_pool(name="pss", bufs=1, space="PSUM"))
        ps_tp = ctx.enter_context(tc.tile_pool(name="pstp", bufs=1, space="PSUM"))

        # ---- DMAs: 4 big need-ordered transfers, parallel queues --------
        cst = sb.tile([128, 19], F32, tag="cst")
        nc.sync.dma_start(cst[:], d_cst[:])
        xin = sb.tile([128, 2560], F8, tag="xin")
        nc.scalar.dma_start(xin[:], d_xin[:])
        yin = sb.tile([128, 4096], F8, tag="yin")
        nc.scalar.dma_start(yin[:], d_yin[:])
        tl = sb.tile([128, 2688], BF16, tag="tl")
        nc.scalar.dma_start(tl[:], d_tail[:])
        vb, notm, ident = tl[:, 0:2048], tl[:, 2048:2560], tl[:, 2560:2688]

        # ---- dummy act on garbage: trigger the ACT_TABLE_LOAD early -----
        dum = sb.tile([128, 1], F32, tag="dum")
        dsrc = sb.tile([128, 1], F32, tag="dsrc")
        nc.vector.memset(dsrc[:], 0.0)
        nc.scalar.activation(dum[:], dsrc[:], AF.Tanh, bias=dsrc[:, 0:1])

        # ---- qWT [a, q] in PSUM -----------------------------------------
        qwt = ps_q.tile([128, 512], F32, tag="qwt")
        xw = xin[:, 0:2048].rearrange("p (h a) -> p h a", h=4)
        xq = xin[:, 2048:2560].rearrange("p (h q) -> p h q", h=4)
        for ab in range(4):
            for hp in range(2):
                nc.tensor.matmul(
                    qwt[:, ab * 128:(ab + 1) * 128],
                    xw[:, 2 * hp:2 * hp + 2, ab * 128:(ab + 1) * 128],
                    xq[:, 2 * hp:2 * hp + 2, :],
                    start=(hp == 0), stop=(hp == 1),
                    perf_mode=mybir.MatmulPerfMode.DoubleRow)

        # ---- x0 = bf16(qwt + b1) ----------------------------------------
        x0 = sb.tile([128, 512], BF16, tag="x0")
        for ab in range(4):
            sl = slice(ab * 128, (ab + 1) * 128)
            nc.vector.tensor_scalar(x0[:, sl], qwt[:, sl],
                                    float(1.0 / 32.0), cst[:, ab:ab + 1],
                                    OP.mult, OP.add)

        # ---- x-acts ------------------------------------------------------
        xt = {1: x0}
        for i in XUSED:
            t = sb.tile([128, 512], BF16, tag=f"xt{i}")
            nc.scalar.activation(t[:], x0[:], AF.Tanh,
                                 bias=cst[:, XB0 + XUSED.index(i):
                                          XB0 + 1 + XUSED.index(i)],
                                 scale=float(ALPHA))
            xt[i] = t

        # ---- kWT [a, k] in PSUM -----------------------------------------
        kwt = ps_k.tile([128, 2048], F32, tag="kwt")
        yw = yin[:, 0:2048].rearrange("p (h a) -> p h a", h=4)
        yk = yin[:, 2048:4096].rearrange("p (h k) -> p h k", h=4)
        for ab in range(4):
            for hp in range(2):
                nc.tensor.matmul(
                    kwt[:, ab * 512:(ab + 1) * 512],
                    yw[:, 2 * hp:2 * hp + 2, ab * 128:(ab + 1) * 128],
                    yk[:, 2 * hp:2 * hp + 2, :],
                    start=(hp == 0), stop=(hp == 1),
                    perf_mode=mybir.MatmulPerfMode.DoubleRow)

        # ---- y-acts (read PSUM, per-ab chunks for pipelining) -----------
        yt = []
        for j in range(NY):
            ytj = sb.tile([128, 2048], F8, tag=f"yt{j}")
            yt.append(ytj)
        for hf in range(2):
            ksl = slice(hf * 1024, (hf + 1) * 1024)
            for j in range(NY):
                nc.scalar.activation(yt[j][:, ksl], kwt[:, ksl], AF.Tanh,
                                     bias=cst[:, YB0 + j:YB0 + 1 + j],
                                     scale=float(ALPHA / 32.0))

        # ---- combos: Phi_j = sum_i C[i,j] phi_i -------------------------
        phi = []
        first = [True] * NY
        for j in range(NY):
            phj = sb.tile([128, 512], BF16, tag=f"phi{j}")
            phi.append(phj)
        for (i, j, c) in ENTRIES:
            if first[j]:
                nc.vector.tensor_scalar_mul(phi[j][:], xt[i][:], float(c))
                first[j] = False
            else:
                nc.vector.scalar_tensor_tensor(
                    phi[j][:], xt[i][:], float(c), phi[j][:],
                    OP.mult, OP.add)

        # ---- w2-fold: lhs_j = (Phi_j + C0_j) * w2 -----------------------
        lhs = []
        for j in range(NY):
            t = sb.tile([128, 512], F8, tag=f"lhs{j}")
            for ab in range(4):
                sl = slice(ab * 128, (ab + 1) * 128)
                nc.vector.tensor_scalar(t[:, sl], phi[j][:, sl],
                                        float(C0[j]), cst[:, 4 + ab:5 + ab],
                                        OP.add, OP.mult)
            lhs.append(t)

        # ---- pairs matmul: scores [q, k] --------------------------------
        sc0 = ps_s.tile([128, 256], F32, tag="sc0")
        sc1 = ps_s.tile([128, 256], F32, tag="sc1")
        sch = [sc0, sc1]
        lhsv = [lhs[j].rearrange("p (a q) -> p a q", a=4) for j in range(NY)]
        ytv = [yt[j].rearrange("p (a k) -> p a k", a=4) for j in range(NY)]
        for abp in range(2):
            for j in range(NY):
                for kh in range(2):
                    nc.tensor.matmul(
                        sch[kh][:],
                        lhsv[j][:, 2 * abp:2 * abp + 2, :],
                        ytv[j][:, 2 * abp:2 * abp + 2,
                               kh * 256:(kh + 1) * 256],
                        start=(abp == 0 and j == 0),
                        stop=(abp == 1 and j == NY - 1),
                        perf_mode=mybir.MatmulPerfMode.DoubleRow)

        # ---- masked softmax ---------------------------------------------
        wexp = sb.tile([128, 512], BF16, tag="wexp")
        wm = sb.tile([128, 512], BF16, tag="wm")
        ssum0 = sb.tile([128, 1], F32, tag="ssum0")
        ssum1 = sb.tile([128, 1], F32, tag="ssum1")
        for kh in range(2):
            ksl = slice(kh * 256, (kh + 1) * 256)
            nc.scalar.activation(wexp[:, ksl], sch[kh][:], AF.Exp,
                                 bias=cst[:, 8:9], scale=float(1.0 / 16.0))
            nc.vector.scalar_tensor_tensor(
                wm[:, ksl], wexp[:, ksl], 1.0, notm[:, ksl],
                OP.mult, OP.mult,
                accum_out=(ssum0[:] if kh == 0 else ssum1[:]))
        ssum = sb.tile([128, 1], F32, tag="ssum")
        nc.vector.tensor_add(ssum[:], ssum0[:], ssum1[:])
        rinv = sb.tile([128, 1], F32, tag="rinv")
        nc.vector.reciprocal(rinv[:], ssum[:])

        # ---- context: (wm^T @ values) * rinv ----------------------------
        wT = sb.tile([128, 512], BF16, tag="wT")
        pt = ps_tp.tile([128, 512], BF16, tag="tp")
        ctxp = ps_q.tile([128, 512], F32, tag="qwt")
        for i in range(4):
            nc.tensor.transpose(pt[:, i * 128:(i + 1) * 128],
                                wm[:, i * 128:(i + 1) * 128], ident)
        for h in range(2):
            nc.vector.tensor_copy(wT[:, h * 256:(h + 1) * 256],
                                  pt[:, h * 256:(h + 1) * 256])
        wout = sb.tile([128, 512], BF16, tag="wout")
        nc.vector.tensor_scalar_mul(wout[:], wm[:], rinv[:])
        nc.sync.dma_start(d_wout[:], wout[:])
        for kc in range(4):
            nc.tensor.matmul(ctxp[:], wT[:, kc * 128:(kc + 1) * 128],
                             tl[:, kc * 512:(kc + 1) * 512],
                             start=(kc == 0), stop=(kc == 3))
        cout = sb.tile([128, 512], BF16, tag="cout")
        for h in range(2):
            csl = slice(h * 256, (h + 1) * 256)
            nc.vector.tensor_scalar_mul(cout[:, csl], ctxp[:, csl], rinv[:])
            eng = nc.sync if h == 0 else nc.scalar
            eng.dma_start(d_cout[:, csl], cout[:, csl])

    nc.compile()
    return nc


_NC_CACHE = None


def _get_nc():
    global _NC_CACHE
    if _NC_CACHE is None:
        _NC_CACHE = _build_kernel()
    return _NC_CACHE


def _host_inputs(query, keys, values, mask, W1, b1, w2, b2):
    query = np.asarray(query, np.float32).astype(NPF8)
    keys = np.asarray(keys, np.float32).astype(NPF8)
    values = np.asarray(values, np.float32).astype(NPBF)
    notm = (~np.asarray(mask, bool)).astype(NPBF)
    W1 = (np.asarray(W1, np.float32) * 32.0).astype(NPF8)
    b1 = np.asarray(b1, np.float32)
    w2 = np.asarray(w2, np.float32)

    cst = np.zeros((128, 19), np.float32)
    cst[:, 0:4] = b1.reshape(4, 128).T
    cst[:, 4:8] = 16.0 * w2.reshape(4, 128).T
    for n, i in enumerate(XUSED):
        cst[:, XB0 + n] = -ALPHA * XMUS[i]
    for j, mu in enumerate(YMUS):
        cst[:, YB0 + j] = -ALPHA * mu

    W1a, W1b = W1[:H], W1[H:]
    w1a = np.concatenate([W1a[hc * 128:(hc + 1) * 128] for hc in range(4)], 1)
    w1b = np.concatenate([W1b[hc * 128:(hc + 1) * 128] for hc in range(4)], 1)

    in_maps = []
    for c in range(N_CORES):
        b, qh = c // 2, c % 2
        qT = np.ascontiguousarray(query[b, qh * QSH:(qh + 1) * QSH, :].T)
        qtc = np.concatenate([qT[hc * 128:(hc + 1) * 128] for hc in range(4)], 1)
        kT = np.ascontiguousarray(keys[b].T)
        ktc = np.concatenate([kT[hc * 128:(hc + 1) * 128] for hc in range(4)], 1)
        vc = np.concatenate([values[b][kc * 128:(kc + 1) * 128]
                             for kc in range(4)], 1)
        xin = np.concatenate([w1a, qtc], 1)
        yin = np.concatenate([w1b, ktc], 1)
        tlc = np.concatenate(
            [vc, notm[b, qh * QSH:(qh + 1) * QSH, :], np.eye(128, dtype=NPBF)], 1)
        in_maps.append({
            "xin": np.ascontiguousarray(xin),
            "cst": cst,
            "yin": np.ascontiguousarray(yin),
            "tl": np.ascontiguousarray(tlc),
        })
    return in_maps


def _run(inputs, trace=False, **kw):
    nc = _get_nc()
    in_maps = _host_inputs(**inputs)
    res = run_bass_kernel_spmd(nc, in_maps, list(range(N_CORES)),
                               trace=trace, **kw)
    context = np.zeros((B, Q, H), np.float32)
    weights = np.zeros((B, Q, K), np.float32)
    for c in range(N_CORES):
        b, qh = c // 2, c % 2
        weights[b, qh * QSH:(qh + 1) * QSH, :] = \
            res.results[c]["wout"].astype(np.float32)
        context[b, qh * QSH:(qh + 1) * QSH, :] = \
            res.results[c]["cout"].astype(np.float32)
    return (context, weights), res


def kernel(query, keys, values, mask, W1, b1, w2, b2):
    (context, weights), _ = _run(dict(query=query, keys=keys, values=values,
                                      mask=mask, W1=W1, b1=b1, w2=w2, b2=b2))
    return context, weights


# revision 4
# speedup vs baseline: 1.1745x; 1.1745x over previous
"""Bahdanau attention kernel v2 for 8 TRN2 NeuronCores.

Math: scores[q,k] = w2 . tanh(qW[q,:] + b1 + kW[k,:])  (b2 dropped, softmax-
invariant).  tanh(x+y) is approximated by a collapsed separable expansion

    tanh(x+y) ~= g(x) + sum_j Phi_j(x) * tanh(1.5*(y - mu_j)),   j = 1..4

where Phi_j = sum_i C[i,j] * phi_i(x) are linear combos (computed on the
VectorE) of a small x-dictionary {x, tanh(1.5(x-mu))} evaluated by the
ScalarE.  g(x) is softmax-invariant and dropped.  All activations (tanh,
exp) live in the single `exp_and_others` table set: one ACT_TABLE_LOAD,
triggered by a dummy act at t=0 so it overlaps the input DMA.

scores = sum_j (w2*Phi_j)^T @ psi_j is a TensorE contraction (16 matmuls),
followed by exp-softmax (mask via host-sent 1-mask) and the context matmul
on the transposed normalized weights.

Sharding: data-parallel, core = (batch b, query-half qh); each core computes
a [128, 512] block of weights and context.  Outputs are bf16, upcast on host.
"""

import numpy as np
import ml_dtypes

from contextlib import ExitStack
from concourse import bass, bacc, tile, mybir
from concourse.bass_utils import run_bass_kernel_spmd

BF16 = mybir.dt.bfloat16
F32 = mybir.dt.float32
AF = mybir.ActivationFunctionType
OP = mybir.AluOpType
NPBF = ml_dtypes.bfloat16
F8 = mybir.dt.float8e4
NPF8 = ml_dtypes.float8_e4m3

B, Q, K, H, A = 4, 256, 512, 512, 512
QSH = 128
N_CORES = 8

ALPHA = 1.5
# x-dictionary: index -> spec. 0='one' (folded), 1='lin' (x0 itself),
# 2..8 = tanh(1.5*(x-mu)), mus:
XMUS = {2: -0.9, 3: -0.45, 4: 0.0, 5: 0.45, 6: 0.9}
YMUS = [-0.8, 0.0, 0.8]
NY = len(YMUS)
# C[0, j] ('one' row): folded into the w2-fold pass
C0 = [0.17376022026341648, -0.08117130489640326, 0.2138754878123161]
# (x_index, y_col, coeff)
ENTRIES = [
    (5, 0, 0.8101097519208896),
    (6, 0, -0.6013097045398987),
    (3, 1, 0.519418159350108),
    (5, 1, -0.5298784785625438),
    (1, 2, 0.05761974496291286),
    (2, 2, 0.5656294633784432),
    (3, 2, -0.8359265955681372),
]
XUSED = sorted({i for i, _, _ in ENTRIES if i >= 2})
XB0 = 9
YB0 = XB0 + len(XUSED)


def _build_kernel():
    nc = bacc.Bacc("TRN2", target_bir_lowering=False, debug=False,
                   num_devices=N_CORES)

    d_xin = nc.declare_dram_parameter("xin", [128, 2560], F8, isOutput=False)
    d_cst = nc.declare_dram_parameter("cst", [128, 19], F32, isOutput=False)
    d_yin = nc.declare_dram_parameter("yin", [128, 4096], F8, isOutput=False)
    d_tail = nc.declare_dram_parameter("tl", [128, 2688], BF16, isOutput=False)
    d_wout = nc.declare_dram_parameter("wout", [QSH, K], BF16, isOutput=True)
    d_cout = nc.declare_dram_parameter("cout", [QSH, H], BF16, isOutput=True)

    with tile.TileContext(nc) as tc, ExitStack() as ctx:
        sb = ctx.enter_context(tc.tile_pool(name="sb", bufs=1))
        ps_q = ctx.enter_context(tc.tile_pool(name="psq", bufs=1, space="PSUM"))
        ps_k = ctx.enter_context(tc.tile_pool(name="psk", bufs=1, space="PSUM"))
        ps_s = ctx.enter_context(tc.tile_pool(name="pss", bufs=1, space="PSUM"))
        ps_tp = ctx.enter_context(tc.tile_pool(name="pstp", bufs=1, space="PSUM"))

        # ---- DMAs: 4 big need-ordered transfers, parallel queues --------
        cst = sb.tile([128, 19], F32, tag="cst")
        nc.sync.dma_start(cst[:], d_cst[:])
        xin = sb.tile([128, 2560], F8, tag="xin")
        nc.scalar.dma_start(xin[:], d_xin[:])
        yin = sb.tile([128, 4096], F8, tag="yin")
        nc.scalar.dma_start(yin[:, 0:3072], d_yin[:, 0:3072])
        nc.scalar.dma_start(yin[:, 3072:4096], d_yin[:, 3072:4096])
        tl = sb.tile([128, 2688], BF16, tag="tl")
        nc.scalar.dma_start(tl[:], d_tail[:])
        vb, notm, ident = tl[:, 0:2048], tl[:, 2048:2560], tl[:, 2560:2688]

        # ---- dummy act on garbage: trigger the ACT_TABLE_LOAD early -----
        dum = sb.tile([128, 1], F32, tag="dum")
        dsrc = sb.tile([128, 1], F32, tag="dsrc")
        nc.vector.memset(dsrc[:], 0.0)
        nc.scalar.activation(dum[:], dsrc[:], AF.Tanh, bias=dsrc[:, 0:1])

        # ---- qWT [a, q] in PSUM -----------------------------------------
        qwt = ps_q.tile([128, 512], F32, tag="qwt")
        xw = xin[:, 0:2048].rearrange("p (h a) -> p h a", h=4)
        xq = xin[:, 2048:2560].rearrange("p (h q) -> p h q", h=4)
        for ab in range(4):
            for hp in range(2):
                nc.tensor.matmul(
                    qwt[:, ab * 128:(ab + 1) * 128],
                    xw[:, 2 * hp:2 * hp + 2, ab * 128:(ab + 1) * 128],
                    xq[:, 2 * hp:2 * hp + 2, :],
                    start=(hp == 0), stop=(hp == 1),
                    perf_mode=mybir.MatmulPerfMode.DoubleRow)

        # ---- x0 = bf16(qwt + b1) ----------------------------------------
        x0 = sb.tile([128, 512], BF16, tag="x0")
        for ab in range(4):
            sl = slice(ab * 128, (ab + 1) * 128)
            nc.vector.tensor_scalar(x0[:, sl], qwt[:, sl],
                                    float(1.0 / 32.0), cst[:, ab:ab + 1],
                                    OP.mult, OP.add)

        # ---- x-acts ------------------------------------------------------
        xt = {1: x0}
        for i in XUSED:
            t = sb.tile([128, 512], BF16, tag=f"xt{i}")
            nc.scalar.activation(t[:], x0[:], AF.Tanh,
                                 bias=cst[:, XB0 + XUSED.index(i):
                                          XB0 + 1 + XUSED.index(i)],
                                 scale=float(ALPHA))
            xt[i] = t

        # ---- kWT [a, k] in PSUM -----------------------------------------
        kwt = ps_k.tile([128, 2048], F32, tag="kwt")
        yw01 = yin[:, 0:1024].rearrange("p (h a) -> p h a", h=4)
        yw23 = yin[:, 3072:4096].rearrange("p (h a) -> p h a", h=4)
        yk = yin[:, 1024:3072].rearrange("p (h k) -> p h k", h=4)
        for ab in range(4):
            yw = yw01 if ab < 2 else yw23
            aoff = (ab % 2) * 128
            for hp in range(2):
                nc.tensor.matmul(
                    kwt[:, ab * 512:(ab + 1) * 512],
                    yw[:, 2 * hp:2 * hp + 2, aoff:aoff + 128],
                    yk[:, 2 * hp:2 * hp + 2, :],
                    start=(hp == 0), stop=(hp == 1),
                    perf_mode=mybir.MatmulPerfMode.DoubleRow)

        # ---- y-acts (read PSUM, per-ab chunks for pipelining) -----------
        yt = []
        for j in range(NY):
            ytj = sb.tile([128, 2048], F8, tag=f"yt{j}")
            yt.append(ytj)
        for hf in range(2):
            ksl = slice(hf * 1024, (hf + 1) * 1024)
            for j in range(NY):
                nc.scalar.activation(yt[j][:, ksl], kwt[:, ksl], AF.Tanh,
                                     bias=cst[:, YB0 + j:YB0 + 1 + j],
                                     scale=float(ALPHA / 32.0))

        # ---- combos: Phi_j = sum_i C[i,j] phi_i -------------------------
        phi = []
        first = [True] * NY
        for j in range(NY):
            phj = sb.tile([128, 512], BF16, tag=f"phi{j}")
            phi.append(phj)
        for (i, j, c) in ENTRIES:
            if first[j]:
                nc.vector.tensor_scalar_mul(phi[j][:], xt[i][:], float(c))
                first[j] = False
            else:
                nc.vector.scalar_tensor_tensor(
                    phi[j][:], xt[i][:], float(c), phi[j][:],
                    OP.mult, OP.add)

        # ---- w2-fold: lhs_j = (Phi_j + C0_j) * w2 -----------------------
        lhs = []
        for j in range(NY):
            t = sb.tile([128, 512], F8, tag=f"lhs{j}")
            for ab in range(4):
                sl = slice(ab * 128, (ab + 1) * 128)
                nc.vector.tensor_scalar(t[:, sl], phi[j][:, sl],
                                        float(C0[j]), cst[:, 4 + ab:5 + ab],
                                        OP.add, OP.mult)
            lhs.append(t)

        # ---- pairs matmul: scores [q, k] --------------------------------
        sc0 = ps_s.tile([128, 256], F32, tag="sc0")
        sc1 = ps_s.tile([128, 256], F32, tag="sc1")
        sch = [sc0, sc1]
        lhsv = [lhs[j].rearrange("p (a q) -> p a q", a=4) for j in range(NY)]
        ytv = [yt[j].rearrange("p (a k) -> p a k", a=4) for j in range(NY)]
        for abp in range(2):
            for j in range(NY):
                for kh in range(2):
                    nc.tensor.matmul(
                        sch[kh][:],
                        lhsv[j][:, 2 * abp:2 * abp + 2, :],
                        ytv[j][:, 2 * abp:2 * abp + 2,
                               kh * 256:(kh + 1) * 256],
                        start=(abp == 0 and j == 0),
                        stop=(abp == 1 and j == NY - 1),
                        perf_mode=mybir.MatmulPerfMode.DoubleRow)

        # ---- masked softmax ---------------------------------------------
        wexp = sb.tile([128, 512], BF16, tag="wexp")
        wm = sb.tile([128, 512], BF16, tag="wm")
        ssum0 = sb.tile([128, 1], F32, tag="ssum0")
        ssum1 = sb.tile([128, 1], F32, tag="ssum1")
        for kh in range(2):
            ksl = slice(kh * 256, (kh + 1) * 256)
            nc.scalar.activation(wexp[:, ksl], sch[kh][:], AF.Exp,
                                 bias=cst[:, 8:9], scale=float(1.0 / 16.0))
            nc.vector.scalar_tensor_tensor(
                wm[:, ksl], wexp[:, ksl], 1.0, notm[:, ksl],
                OP.mult, OP.mult,
                accum_out=(ssum0[:] if kh == 0 else ssum1[:]))
        ssum = sb.tile([128, 1], F32, tag="ssum")
        nc.vector.tensor_add(ssum[:], ssum0[:], ssum1[:])
        rinv = sb.tile([128, 1], F32, tag="rinv")
        nc.vector.reciprocal(rinv[:], ssum[:])

        # ---- context: (wm^T @ values) * rinv ----------------------------
        wT = sb.tile([128, 512], BF16, tag="wT")
        pt = ps_tp.tile([128, 512], BF16, tag="tp")
        ctxp = ps_q.tile([128, 512], F32, tag="qwt")
        for i in range(4):
            nc.tensor.transpose(pt[:, i * 128:(i + 1) * 128],
                                wm[:, i * 128:(i + 1) * 128], ident)
        for h in range(2):
            nc.vector.tensor_copy(wT[:, h * 256:(h + 1) * 256],
                                  pt[:, h * 256:(h + 1) * 256])
        wout = sb.tile([128, 512], BF16, tag="wout")
        nc.vector.tensor_scalar_mul(wout[:], wm[:], rinv[:])
        nc.sync.dma_start(d_wout[:], wout[:])
        for kc in range(4):
            nc.tensor.matmul(ctxp[:], wT[:, kc * 128:(kc + 1) * 128],
                             tl[:, kc * 512:(kc + 1) * 512],
                             start=(kc == 0), stop=(kc == 3))
        cout = sb.tile([128, 512], BF16, tag="cout")
        for h in range(2):
            csl = slice(h * 256, (h + 1) * 256)
            nc.vector.tensor_scalar_mul(cout[:, csl], ctxp[:, csl], rinv[:])
            eng = nc.sync if h == 0 else nc.scalar
            eng.dma_start(d_cout[:, csl], cout[:, csl])

    nc.compile()
    return nc


_NC_CACHE = None


def _get_nc():
    global _NC_CACHE
    if _NC_CACHE is None:
        _NC_CACHE = _build_kernel()
    return _NC_CACHE


def _host_inputs(query, keys, values, mask, W1, b1, w2, b2):
    query = np.asarray(query, np.float32).astype(NPF8)
    keys = np.asarray(keys, np.float32).astype(NPF8)
    values = np.asarray(values, np.float32).astype(NPBF)
    notm = (~np.asarray(mask, bool)).astype(NPBF)
    W1 = (np.asarray(W1, np.float32) * 32.0).astype(NPF8)
    b1 = np.asarray(b1, np.float32)
    w2 = np.asarray(w2, np.float32)

    cst = np.zeros((128, 19), np.float32)
    cst[:, 0:4] = b1.reshape(4, 128).T
    cst[:, 4:8] = 16.0 * w2.reshape(4, 128).T
    for n, i in enumerate(XUSED):
        cst[:, XB0 + n] = -ALPHA * XMUS[i]
    for j, mu in enumerate(YMUS):
        cst[:, YB0 + j] = -ALPHA * mu

    W1a, W1b = W1[:H], W1[H:]
    w1a = np.concatenate([W1a[hc * 128:(hc + 1) * 128] for hc in range(4)], 1)
    w1b = np.concatenate([W1b[hc * 128:(hc + 1) * 128] for hc in range(4)], 1)

    in_maps = []
    for c in range(N_CORES):
        b, qh = c // 2, c % 2
        qT = np.ascontiguousarray(query[b, qh * QSH:(qh + 1) * QSH, :].T)
        qtc = np.concatenate([qT[hc * 128:(hc + 1) * 128] for hc in range(4)], 1)
        kT = np.ascontiguousarray(keys[b].T)
        ktc = np.concatenate([kT[hc * 128:(hc + 1) * 128] for hc in range(4)], 1)
        vc = np.concatenate([values[b][kc * 128:(kc + 1) * 128]
                             for kc in range(4)], 1)
        xin = np.concatenate([w1a, qtc], 1)
        w1b01 = np.concatenate([W1b[hc * 128:(hc + 1) * 128, 0:256]
                                for hc in range(4)], 1)
        w1b23 = np.concatenate([W1b[hc * 128:(hc + 1) * 128, 256:512]
                                for hc in range(4)], 1)
        yin = np.concatenate([w1b01, ktc, w1b23], 1)
        tlc = np.concatenate(
            [vc, notm[b, qh * QSH:(qh + 1) * QSH, :], np.eye(128, dtype=NPBF)], 1)
        in_maps.append({
            "xin": np.ascontiguousarray(xin),
            "cst": cst,
            "yin": np.ascontiguousarray(yin),
            "tl": np.ascontiguousarray(tlc),
        })
    return in_maps


def _run(inputs, trace=False, **kw):
    nc = _get_nc()
    in_maps = _host_inputs(**inputs)
    res = run_bass_kernel_spmd(nc, in_maps, list(range(N_CORES)),
                               trace=trace, **kw)
    context = np.zeros((B, Q, H), np.float32)
    weights = np.zeros((B, Q, K), np.float32)
    for c in range(N_CORES):
        b, qh = c // 2, c % 2
        weights[b, qh * QSH:(qh + 1) * QSH, :] = \
            res.results[c]["wout"].astype(np.float32)
        context[b, qh * QSH:(qh + 1) * QSH, :] = \
            res.results[c]["cout"].astype(np.float32)
    return (context, weights), res


def kernel(query, keys, values, mask, W1, b1, w2, b2):
    (context, weights), _ = _run(dict(query=query, keys=keys, values=values,
                                      mask=mask, W1=W1, b1=b1, w2=w2, b2=b2))
    return context, weights
